# revision 26
# baseline (speedup 1.0000x reference)
"""Distributed Trainium2 kernel for GQA attention (B=2, T=2048, D=2048, N=8
query heads, K=1 KV head, H=256) on 8 NeuronCores.

Sharding (causal fast path): 2 (batch) x 4 (sequence) mesh with NO collectives.
Core c = 4*b + r handles batch b and query chunks {r, 4+r, 8+r, 12+r} (128
tokens each; strided assignment balances causal work exactly). Each core
computes K/V over the full sequence (replicated within the batch group), all 8
heads of attention for its 512 query tokens, and the full out-projection for
those rows -- the head sum is local, so each core DMAs out its own 512 rows.

The graph is identical on all 8 cores (single NEFF). Per-core differences are
pure data:
  - x columns are rolled by -128*r inside each 512 block so the owned query
    chunks sit at physical positions {0, 512, 1024, 1536}; K/V inherit the
    rolled order and the causal structure only depends on the group index
    l//4, which the roll preserves.
  - the partially/fully masked "band" tiles (key chunk in the same group-of-4
    as the query block) are 0/1 data tiles: for physical band slot k',
    logical k = (k'+r)%4: k<r all-ones, k==r diagonal, k>r all-zeros.

Device layout (transposed attention, all bf16):
  xT [D, T] rolled; qT per head [256, 512-owned]; kT [256, T]; v [T, 256]
  logitsT [key-chunk 128, q-suffix w] where w = 512 - 128*(chunk//4) -- the
  suffix structure computes exactly the causally-needed chunks (+ the band
  slack), uniformly across cores.
  exp via ScalarE (bias -2 keeps exp in range without a max pass), dsum via
  ones-column matmul, reciprocal + partition broadcast, normalize into enc,
  out[tok 128, D-blk 512] = sum over 16 enc chunks @ wo.

The dense/masked variants keep the legacy 2x4 head-parallel kernel with the
chunked ReduceScatter (below).
"""

import numpy as np
import ml_dtypes

import concourse.bass as bass
import concourse.bacc as bacc
import concourse.mybir as mybir
import concourse.tile as tile
from concourse import bass_utils

BF = mybir.dt.bfloat16
F32 = mybir.dt.float32

B, T, D, N, KVH, H = 2, 2048, 2048, 8, 1, 256
MAX_WAVELENGTH = 10000
TBLK = 512    # T block (matmul moving free dim / PSUM bank)
SCH = 128     # S chunk (key chunk, PSUM partition dim)
TT = 128      # T tile (out-projection partition dim)
RS_ROWS = 256  # rows per ReduceScatter chunk (legacy)
GROUPS = [[0, 1, 2, 3], [4, 5, 6, 7]]
N_CORES = 8
NDCH = D // 128   # 16 D chunks
NQB = 4           # owned query blocks per core
QW = NQB * SCH    # 512 owned query columns


# --------------------------------------------------------------------------
# causal fast path: sequence-parallel, no collectives
# --------------------------------------------------------------------------

def build_fast():
    nc = bacc.Bacc("TRN2", target_bir_lowering=False, debug=False,
                   num_devices=N_CORES)

    # all weight/activation inputs are host-prearranged partition-major
    # ([128, chunks*cols]) so each load is one contiguous segment per
    # partition -- 128 DMA descriptors instead of 1-2k, which keeps the
    # Sync engine's dispatch off the critical path
    xown_e = nc.dram_tensor("xown", [128, NDCH * TBLK], BF, kind="ExternalInput")
    xq_e = nc.dram_tensor("xq", [128, NDCH * QW], BF, kind="ExternalInput")
    wq_e = nc.dram_tensor("wq", [128, 4 * NDCH * 2 * H], BF, kind="ExternalInput")
    wk_e = nc.dram_tensor("wk", [128, NDCH * H], BF, kind="ExternalInput")
    wv_e = nc.dram_tensor("wv", [128, NDCH * H], BF, kind="ExternalInput")
    wo_e = nc.dram_tensor("wo", [128, NDCH * D], BF, kind="ExternalInput")
    cosq_e = nc.dram_tensor("cosq", [H // 2, QW], BF, kind="ExternalInput")
    sinq_e = nc.dram_tensor("sinq", [H // 2, QW], BF, kind="ExternalInput")
    cosk_e = nc.dram_tensor("cosk", [H // 2, TBLK], BF, kind="ExternalInput")
    sink_e = nc.dram_tensor("sink", [H // 2, TBLK], BF, kind="ExternalInput")
    cm_e = nc.dram_tensor("cmask", [SCH, 4 * SCH], BF, kind="ExternalInput")
    out_e = nc.dram_tensor("out", [QW, D], BF, kind="ExternalOutput")

    with tile.TileContext(nc) as tc:
        poolP = tc.alloc_tile_pool(name="persist", bufs=1)
        poolT = tc.alloc_tile_pool(name="tmps", bufs=6)
        poolPS = tc.alloc_tile_pool(name="ps", bufs=1, space="PSUM")
        poolD = tc.alloc_tile_pool(name="dram", bufs=1, space="DRAM")
        poolW = tc.alloc_tile_pool(name="w", bufs=1)

        # ---- input SBUF tiles -------------------------------------------
        xo_sb = poolW.tile([128, NDCH * TBLK], BF, name="xo_sb")
        xq_sb = poolW.tile([128, NDCH * QW], BF, name="xq_sb")
        wq_sb = poolW.tile([128, NDCH * N * H], BF, name="wq_sb")
        wk_sb = poolW.tile([128, NDCH * H], BF, name="wk_sb")
        wv_sb = poolW.tile([128, NDCH * H], BF, name="wv_sb")
        cosq_sb = poolW.tile([128, QW], BF, name="cosq_sb")
        sinq_sb = poolW.tile([128, QW], BF, name="sinq_sb")
        cosk_sb = poolW.tile([128, TBLK], BF, name="cosk_sb")
        sink_sb = poolW.tile([128, TBLK], BF, name="sink_sb")

        xo3 = xo_sb.rearrange("p (i t) -> p i t", i=NDCH)
        xq3 = xq_sb.rearrange("p (i t) -> p i t", i=NDCH)
        # wq is quarter-major: [p, head-pair h2, dchunk i, 512 cols (2 heads)]
        wq4 = wq_sb.rearrange("p (h2 i c) -> p h2 i c", h2=4, i=NDCH)
        wk3 = wk_sb.rearrange("p (i c) -> p i c", i=NDCH)
        wv3 = wv_sb.rearrange("p (i c) -> p i c", i=NDCH)
        QSZ = NDCH * 2 * H

        # tiny warmup AllGather: the first collective pays the cold-start
        # trigger delay (~12-29us) plus a ~15us fixed execution floor; fire
        # one with NO dependencies (gathers uninitialized DRAM, output
        # unused) so all of that burns at t=0 under the input DMAs instead
        # of on the real K/V gather
        wag_in = poolD.tile([1, 64], BF, name="wag_in")
        wag_out = poolD.tile([4, 64], BF, name="wag_out")
        nc.gpsimd.collective_compute(
            "AllGather", mybir.AluOpType.bypass,
            replica_groups=GROUPS,
            ins=[wag_in[:].opt()],
            outs=[wag_out[:].opt()])

        # own-block K/V proj inputs go first; Q proj inputs right behind.
        # each tensor is split into several dma_starts -- a single transfer
        # tops out at ~150GB/s on one queue, splits run in parallel
        def load_split(dst, src_ap, n):
            tot = dst.shape[-1]
            step = tot // n
            for s in range(n):
                sl = slice(s * step, (s + 1) * step)
                nc.sync.dma_start(dst[:, sl], src_ap[:, sl])

        load_split(wk_sb, wk_e.ap(), 2)
        load_split(xo_sb, xown_e.ap(), 4)
        load_split(wv_sb, wv_e.ap(), 2)
        load_split(xq_sb, xq_e.ap(), 4)
        load_split(wq_sb[:, 0:QSZ], wq_e.ap()[:, 0:QSZ], 2)
        nc.sync.dma_start(cosk_sb[:], cosk_e.ap()[:, :])
        nc.sync.dma_start(sink_sb[:], sink_e.ap()[:, :])
        nc.sync.dma_start(cosq_sb[:], cosq_e.ap()[:, :])
        nc.sync.dma_start(sinq_sb[:], sinq_e.ap()[:, :])
        cm_sb = poolP.tile([SCH, 4 * SCH], BF, name="cm_sb")
        nc.sync.dma_start(cm_sb[:], cm_e.ap()[:, :])
        for qq in range(1, 4):
            load_split(wq_sb[:, qq * QSZ:(qq + 1) * QSZ],
                       wq_e.ap()[:, qq * QSZ:(qq + 1) * QSZ], 2)

        ones_col = poolP.tile([128, 1], BF, name="ones_col")
        nc.vector.memset(ones_col[:], 1.0)
        # warm the gpsimd partition_broadcast library while DMAs stream so
        # the first real broadcast doesn't pay the ~12us LOAD_LIB on the
        # norm critical path
        warm1 = poolP.tile([1, 8], F32, name="warm1")
        warm2 = poolP.tile([128, 8], F32, name="warm2")
        nc.vector.memset(warm1[:], 1.0)
        nc.gpsimd.partition_broadcast(warm2[:], warm1[:])

        # persistent homes for attention-phase outputs so they don't alias
        # the phase-A weight pool (aliasing would stall head 0's norm until
        # every weight read completes)
        enc_sb = poolP.tile([128, NDCH * QW], BF, name="enc_sb")
        enc3 = enc_sb.rearrange("p (i c) -> p i c", i=NDCH)

        # ---- persistent activation tiles --------------------------------
        k_sb = poolP.tile([128, 2 * T], BF, name="k_sb")      # [top|bot]
        v_sb = poolP.tile([128, (T // SCH) * H], BF, name="v_sb")
        q_all = poolP.tile([128, N * 2 * QW], BF, name="q_all")
        v3 = v_sb.rearrange("p (j c) -> p j c", j=T // SCH)
        k_own = poolP.tile([128, 2 * TBLK], BF, name="k_own")
        v_own = poolP.tile([128, 4 * H], BF, name="v_own")

        def qtop(h):
            return q_all[:, h * 2 * QW:h * 2 * QW + QW]

        def qbot(h):
            return q_all[:, h * 2 * QW + QW:(h + 1) * 2 * QW]

        # ---- phase A: own-block K/V + AllGather, Q projections + rope ---
        def emit_qproj(h):
            ps_qt = poolPS.tile([128, QW], F32, name="ps_qt", tag="qk", bufs=3)
            ps_qb = poolPS.tile([128, QW], F32, name="ps_qb", tag="qk", bufs=3)
            h2, ho = h // 2, (h % 2) * H
            for di in range(NDCH):
                nc.tensor.matmul(ps_qt[:], wq4[:, h2, di, ho:ho + 128],
                                 xq3[:, di, :], start=(di == 0),
                                 stop=(di == NDCH - 1))
            for di in range(NDCH):
                nc.tensor.matmul(ps_qb[:], wq4[:, h2, di, ho + 128:ho + H],
                                 xq3[:, di, :], start=(di == 0),
                                 stop=(di == NDCH - 1))
            return ("q", h, ps_qt, ps_qb)

        def emit_kproj_own():
            ps_kt = poolPS.tile([128, TBLK], F32, name="ps_kt", tag="enc", bufs=3)
            ps_kb = poolPS.tile([128, TBLK], F32, name="ps_kb", tag="enc", bufs=3)
            for di in range(NDCH):
                nc.tensor.matmul(ps_kt[:], wk3[:, di, 0:128], xo3[:, di, :],
                                 start=(di == 0), stop=(di == NDCH - 1))
            for di in range(NDCH):
                nc.tensor.matmul(ps_kb[:], wk3[:, di, 128:256], xo3[:, di, :],
                                 start=(di == 0), stop=(di == NDCH - 1))
            return ("k", 0, ps_kt, ps_kb)

        def emit_vchunk_own(j):
            ps_v = poolPS.tile([128, H], F32, name="ps_v", tag="aux", bufs=2)
            for di in range(NDCH):
                nc.tensor.matmul(ps_v[:], xo3[:, di, j * SCH:(j + 1) * SCH],
                                 wv3[:, di, :], start=(di == 0),
                                 stop=(di == NDCH - 1))
            nc.vector.tensor_copy(v_own[:, j * H:(j + 1) * H], ps_v[:])

        def emit_rope(job):
            kind, idx, ps_t, ps_b = job
            if kind == "q":
                c_sl, s_sl = cosq_sb[:, :], sinq_sb[:, :]
                top_dst, bot_dst = qtop(idx), qbot(idx)
            else:
                c_sl, s_sl = cosk_sb[:, :], sink_sb[:, :]
                top_dst, bot_dst = k_own[:, 0:TBLK], k_own[:, TBLK:2 * TBLK]
            t1 = poolT.tile([128, TBLK], F32, name="rt1", tag="tmp")
            t4 = poolT.tile([128, TBLK], F32, name="rt4", tag="tmp")
            nc.vector.tensor_mul(t1[:], ps_t[:], c_sl)
            nc.vector.tensor_mul(t4[:], ps_t[:], s_sl)
            t2 = poolT.tile([128, TBLK], F32, name="rt2", tag="tmp")
            t3 = poolT.tile([128, TBLK], F32, name="rt3", tag="tmp")
            nc.vector.tensor_mul(t2[:], ps_b[:], s_sl)
            nc.vector.tensor_mul(t3[:], ps_b[:], c_sl)
            nc.vector.tensor_sub(top_dst, t1[:], t2[:])
            nc.vector.tensor_add(bot_dst, t3[:], t4[:])

        # ---- attention logits/exp (pre-emitted for head 0 so the exp
        # pipeline is primed when PV starts) -------------------------------
        # group g covers key chunks 4g..4g+3, all with query-suffix width
        # w = 512-128*g starting at qo = 128*g.
        def emit_le(h, g):
            """logits+exp for the 4 chunks of group g, then the band mask."""
            qo = SCH * g
            w = QW - qo
            ex = poolP.tile([128, 4 * TBLK], BF, name="ex", tag="ex", bufs=4)
            for ci in range(4):
                l = 4 * g + ci
                ps_l = poolPS.tile([128, TBLK], F32, name="ps_l", tag="qk",
                                   bufs=3)
                nc.tensor.matmul(ps_l[:, :w], k_sb[:, l * SCH:(l + 1) * SCH],
                                 qtop(h)[:, qo:], start=True, stop=False)
                nc.tensor.matmul(ps_l[:, :w],
                                 k_sb[:, T + l * SCH:T + (l + 1) * SCH],
                                 qbot(h)[:, qo:], start=False, stop=True)
                nc.scalar.activation(ex[:, ci * TBLK + qo:(ci + 1) * TBLK],
                                     ps_l[:, :w],
                                     mybir.ActivationFunctionType.Exp)
            ex4 = ex.rearrange("p (c x) -> p c x", c=4)
            nc.vector.tensor_mul(ex4[:, :, qo:qo + SCH], ex4[:, :, qo:qo + SCH],
                                 cm_sb.rearrange("p (c x) -> p c x", c=4))
            return (g, qo, w, ex)

        def emit_pv(h, job, ps_e0, ps_e1, ps_ds):
            g, qo, w, ex = job
            for ci in range(4):
                l = 4 * g + ci
                e_sl = slice(ci * TBLK + qo, (ci + 1) * TBLK)
                start = l == 0
                stop = l == 15
                nc.tensor.matmul(ps_e0[:, qo:], v3[:, l, 0:128], ex[:, e_sl],
                                 start=start, stop=stop)
                nc.tensor.matmul(ps_e1[:, qo:], v3[:, l, 128:256], ex[:, e_sl],
                                 start=start, stop=stop)
                nc.tensor.matmul(ps_ds[:, qo:], ones_col[:], ex[:, e_sl],
                                 start=start, stop=stop)

        # ---- phase A execution ------------------------------------------
        pending = emit_kproj_own()
        for j in range(4):
            emit_vchunk_own(j)
        job = emit_qproj(0)
        emit_rope(pending)          # rope own K
        pending = job

        # export roped own K/V, AllGather within the batch group, import
        kv_loc = poolD.tile([128, 2 * TBLK + 4 * H], BF, name="kv_loc")
        kv_all = poolD.tile([4 * 128, 2 * TBLK + 4 * H], BF, name="kv_all")
        nc.sync.dma_start(kv_loc[:, 0:2 * TBLK], k_own[:])
        nc.sync.dma_start(kv_loc[:, 2 * TBLK:], v_own[:])
        nc.gpsimd.collective_compute(
            "AllGather", mybir.AluOpType.bypass,
            replica_groups=GROUPS,
            ins=[kv_loc[:].opt()],
            outs=[kv_all[:].opt()])
        kvr = kv_all.rearrange("(j p) c -> p j c", p=128)
        ktop3 = k_sb[:, 0:T].rearrange("p (j c) -> p j c", j=4)
        kbot3 = k_sb[:, T:2 * T].rearrange("p (j c) -> p j c", j=4)
        for j0 in (0, 2):
            nc.gpsimd.dma_start(ktop3[:, j0:j0 + 2, :],
                                kvr[:, j0:j0 + 2, 0:TBLK])
            nc.gpsimd.dma_start(kbot3[:, j0:j0 + 2, :],
                                kvr[:, j0:j0 + 2, TBLK:2 * TBLK])
        for j in range(4):
            nc.gpsimd.dma_start(v3[:, 4 * j:4 * j + 4, :],
                                kvr[:, j, 2 * TBLK:])

        h0_jobs = []
        for h in range(1, N):
            job = emit_qproj(h)
            emit_rope(pending)
            pending = job
            if h == 6:
                h0_jobs.append(emit_le(0, 0))
            elif h == 7:
                h0_jobs.append(emit_le(0, 1))
        emit_rope(pending)

        poolW.release()
        poolB = tc.alloc_tile_pool(name="phaseB", bufs=1)

        wo_sb = poolB.tile([128, NDCH * D], BF, name="wo_sb")
        wo3 = wo_sb.rearrange("p (i c) -> p i c", i=NDCH)
        wosz = NDCH * D // 4
        for s in range(4):
            nc.sync.dma_start(wo_sb[:, s * wosz:(s + 1) * wosz],
                              wo_e.ap()[:, s * wosz:(s + 1) * wosz])

        # ---- phase B: attention per head --------------------------------
        for h in range(N):
            ps_e0 = poolPS.tile([128, QW], F32, name="ps_e0", tag="enc", bufs=3)
            ps_e1 = poolPS.tile([128, QW], F32, name="ps_e1", tag="enc", bufs=3)
            ps_ds = poolPS.tile([1, QW], F32, name="ps_ds", tag="aux", bufs=2)
            if h == 0:
                exq = h0_jobs
                next_g = len(h0_jobs)
            else:
                exq = [emit_le(h, 0)]
                next_g = 1
            for g in range(4):
                if next_g < 4:
                    exq.append(emit_le(h, next_g))
                    next_g += 1
                emit_pv(h, exq.pop(0), ps_e0, ps_e1, ps_ds)
            rrow = poolP.tile([1, QW], F32, name="rrow", tag="rrow", bufs=2)
            nc.vector.reciprocal_approx_fast(rrow[:], ps_ds[:])
            rbc = poolP.tile([128, QW], F32, name="rbc", tag="rbc", bufs=2)
            nc.gpsimd.partition_broadcast(rbc[:], rrow[:])
            nc.vector.tensor_mul(enc3[:, 2 * h, :], ps_e0[:], rbc[:])
            nc.vector.tensor_mul(enc3[:, 2 * h + 1, :], ps_e1[:], rbc[:])

        # ---- phase C: out-projection ------------------------------------
        for tt in range(NQB):
            t_sl = slice(tt * TT, (tt + 1) * TT)
            for db in range(D // TBLK):
                d_sl = slice(db * TBLK, (db + 1) * TBLK)
                ps_o = poolPS.tile([128, TBLK], F32, name="ps_o", tag="qk",
                                   bufs=3)
                for k in range(NDCH):
                    nc.tensor.matmul(ps_o[:], enc3[:, k, t_sl], wo3[:, k, d_sl],
                                     start=(k == 0), stop=(k == NDCH - 1))
                ostg = poolB.tile([128, TBLK], BF, name="ostg", tag="ostg",
                                  bufs=4)
                nc.vector.tensor_copy(ostg[:], ps_o[:])
                nc.sync.dma_start(out_e.ap()[t_sl, d_sl], ostg[:])

        poolB.release()
        poolD.release()
        poolPS.release()
        poolT.release()
        poolP.release()

    nc.compile()
    return nc


def _rope_tables_cols(pos):
    """pos [ncols] f32 -> cos, sin [H/2, ncols] f32."""
    half = H // 2
    freq_exp = (2.0 / H) * np.arange(half, dtype=np.float32)
    timescale = (MAX_WAVELENGTH ** freq_exp).astype(np.float32)
    radians = pos[None, :].astype(np.float32) / timescale[:, None]
    return np.cos(radians), np.sin(radians)


def _prep_fast_in_maps(x, positions, wq, wkv, wo):
    bf = ml_dtypes.bfloat16
    scale = np.float32(H) ** np.float32(-0.5)
    wq_s = (np.asarray(wq, np.float32) * scale)
    # head-major columns [D, N*H]
    wq_cat = np.ascontiguousarray(
        np.concatenate([wq_s[h] for h in range(N)], axis=1)).astype(bf)
    wk = np.asarray(wkv[0, 0], np.float32).astype(bf)
    wv = np.asarray(wkv[1, 0], np.float32).astype(bf)
    wo_cat = np.ascontiguousarray(
        np.concatenate([np.asarray(wo[h], np.float32) for h in range(N)],
                       axis=0)).astype(bf)

    ds = np.arange(SCH)[:, None]
    dt_ = np.arange(SCH)[None, :]
    diag = (dt_ >= ds).astype(np.float32)
    ones = np.ones((SCH, SCH), np.float32)
    zeros = np.zeros((SCH, SCH), np.float32)

    def pm(a):
        """[D-like, C] -> partition-major [128, NDCH*C] (contiguous per
        partition so the load is 128 DMA descriptors)."""
        dd, cc = a.shape
        return np.ascontiguousarray(
            a.reshape(dd // 128, 128, cc).transpose(1, 0, 2).reshape(128, -1))

    wq_pm = np.concatenate(
        [pm(wq_cat[:, q * 2 * H:(q + 1) * 2 * H]) for q in range(4)],
        axis=1)
    wk_pm, wv_pm, wo_pm = pm(wk), pm(wv), pm(wo_cat)

    in_maps = []
    for c in range(N_CORES):
        b, r = divmod(c, 4)
        xb = np.asarray(x[b], np.float32)          # [T, D]
        pos = np.asarray(positions[b], np.float32)
        # own contiguous block (K/V production) and owned query chunks
        blk = slice(r * TBLK, (r + 1) * TBLK)
        xown = pm(np.ascontiguousarray(xb[blk].T).astype(bf))  # [128, 16*512]
        own = np.concatenate([np.arange((4 * i + r) * SCH,
                                        (4 * i + r + 1) * SCH)
                              for i in range(NQB)])
        xq = pm(np.ascontiguousarray(xb[own].T).astype(bf))
        cosk, sink = _rope_tables_cols(pos[blk])
        cosq, sinq = _rope_tables_cols(pos[own])
        # band tiles: key chunk 4g+k vs query block g (chunk 4g+r):
        # k<r fully visible, k==r diagonal, k>r fully masked
        tiles = [ones if k < r else (diag if k == r else zeros)
                 for k in range(4)]
        cm = np.concatenate(tiles, axis=1).astype(bf)
        in_maps.append({
            "xown": xown, "xq": xq, "wq": wq_pm, "wk": wk_pm, "wv": wv_pm,
            "wo": wo_pm,
            "cosq": cosq.astype(bf), "sinq": sinq.astype(bf),
            "cosk": cosk.astype(bf), "sink": sink.astype(bf), "cmask": cm,
        })
    return in_maps


def _unshard_fast(results):
    out = np.empty((B, T, D), np.float32)
    for c in range(N_CORES):
        b, r = divmod(c, 4)
        shard = np.asarray(results[c]["out"], dtype=np.float32)
        for i in range(NQB):
            t0 = (4 * i + r) * SCH
            out[b, t0:t0 + SCH, :] = shard[i * SCH:(i + 1) * SCH, :]
    return out


# --------------------------------------------------------------------------
# legacy path (dense / masked variants): 2 (batch) x 4 (head-pair) mesh with
# chunked ReduceScatter over the out-projection partial sums.
# --------------------------------------------------------------------------

def rs_regions(t):
    n_tblk = t // TBLK
    regions = [(m * TBLK, TBLK) for m in range(n_tblk - 1)]
    last0 = (n_tblk - 1) * TBLK
    return regions + [(last0, TBLK // 2), (last0 + TBLK // 2, TBLK // 2)]


def build(variant="causal", t=T, d=D):
    """Legacy SPMD graph (identical on all 8 cores)."""
    causal = variant == "causal"
    masked = variant == "masked"
    n_tblk = t // TBLK
    n_dch = d // 128
    n_dblk = d // TBLK
    n_sch = t // SCH
    n_rs = t // RS_ROWS

    nc = bacc.Bacc("TRN2", target_bir_lowering=False, debug=False,
                   num_devices=N_CORES)

    xT_e = nc.dram_tensor("xT", [d, t], BF, kind="ExternalInput")
    wq_e = nc.dram_tensor("wq", [d, 2 * H], BF, kind="ExternalInput")
    wk_e = nc.dram_tensor("wk", [d, H], BF, kind="ExternalInput")
    wv_e = nc.dram_tensor("wv", [d, H], BF, kind="ExternalInput")
    wo_e = nc.dram_tensor("wo", [2 * H, d], BF, kind="ExternalInput")
    cos_e = nc.dram_tensor("cosT", [H // 2, t], F32, kind="ExternalInput")
    sin_e = nc.dram_tensor("sinT", [H // 2, t], F32, kind="ExternalInput")
    if causal:
        cm_e = nc.dram_tensor("cmask", [SCH, 4 * TBLK], BF, kind="ExternalInput")
    if masked:
        gm_e = nc.dram_tensor("gmask", [t, t], BF, kind="ExternalInput")
    out_e = nc.dram_tensor("out", [t // 4, d], BF, kind="ExternalOutput")

    with tile.TileContext(nc) as tc:
        poolP = tc.alloc_tile_pool(name="persist", bufs=1)
        poolT = tc.alloc_tile_pool(name="tmps", bufs=4)
        poolPS = tc.alloc_tile_pool(name="ps", bufs=1, space="PSUM")
        poolD = tc.alloc_tile_pool(name="dram", bufs=1, space="DRAM")
        poolB0 = tc.alloc_tile_pool(name="region0", bufs=1)
        poolW = tc.alloc_tile_pool(name="w", bufs=1)

        x_sb = poolW.tile([128, n_dch * t], BF, name="x_sb")
        wq_sb = poolW.tile([128, n_dch * 2 * H], BF, name="wq_sb")
        wk_sb = poolW.tile([128, n_dch * H], BF, name="wk_sb")
        wv_sb = poolW.tile([128, n_dch * H], BF, name="wv_sb")
        xts = [x_sb[:, i * t:(i + 1) * t] for i in range(n_dch)]
        wqs = [wq_sb[:, i * 2 * H:(i + 1) * 2 * H] for i in range(n_dch)]
        wks = [wk_sb[:, i * H:(i + 1) * H] for i in range(n_dch)]
        wvs = [wv_sb[:, i * H:(i + 1) * H] for i in range(n_dch)]
        wos = [poolP.tile([128, d], BF, name=f"wot{k}") for k in range(4)]
        cos_sb = poolP.tile([128, t], F32, name="cos_sb")
        sin_sb = poolP.tile([128, t], F32, name="sin_sb")

        xT_r = xT_e.ap().rearrange("(i p) t -> p i t", p=128)
        x_sb3 = x_sb.rearrange("p (i t) -> p i t", i=n_dch)

        def load_x_cols(c0, c1, n_split=2):
            step = n_dch // n_split
            for s in range(n_split):
                i0, i1 = s * step, (s + 1) * step
                nc.sync.dma_start(x_sb3[:, i0:i1, c0:c1], xT_r[:, i0:i1, c0:c1])

        def load_w(dst, src, cols, n_split=2):
            src_r = src.ap().rearrange("(i p) c -> p i c", p=128)
            dst_r = dst.rearrange("p (i c) -> p i c", i=n_dch)
            step = n_dch // n_split
            for s in range(n_split):
                i0, i1 = s * step, (s + 1) * step
                nc.sync.dma_start(dst_r[:, i0:i1, :], src_r[:, i0:i1, :])

        load_w(wv_sb, wv_e, H)
        load_x_cols(0, TBLK, n_split=4)
        nc.sync.dma_start(cos_sb[:], cos_e.ap()[:, :])
        nc.sync.dma_start(sin_sb[:], sin_e.ap()[:, :])
        load_w(wk_sb, wk_e, H)
        load_w(wq_sb, wq_e, 2 * H)
        if causal:
            cm_sb = poolP.tile([SCH, 4 * TBLK], BF, name="cm_sb")
            nc.sync.dma_start(cm_sb[:], cm_e.ap()[:, :])
        if n_tblk > 1:
            load_x_cols(TBLK, t, n_split=4)
        for k in range(4):
            nc.sync.dma_start(wos[k][:], wo_e.ap()[128 * k:128 * (k + 1), :])

        ones_col = poolP.tile([128, 1], BF, name="ones_col")
        nc.vector.memset(ones_col[:], 1.0)

        v_sb = [poolP.tile([128, H], BF, name=f"v{j}") for j in range(n_sch)]
        ktop = poolP.tile([128, t], BF, name="ktop")
        kbot = poolP.tile([128, t], BF, name="kbot")
        qtop = [poolP.tile([128, t], BF, name=f"qtop{h}") for h in range(2)]
        qbot = [poolP.tile([128, t], BF, name=f"qbot{h}") for h in range(2)]

        def emit_proj(w_tiles, col0, m):
            sl = slice(m * TBLK, (m + 1) * TBLK)
            ps_top = poolPS.tile([128, TBLK], F32, name="ps_top", tag="qk", bufs=2)
            ps_bot = poolPS.tile([128, TBLK], F32, name="ps_bot", tag="enc", bufs=2)
            for di in range(n_dch):
                nc.tensor.matmul(ps_top[:], w_tiles[di][:, col0:col0 + 128],
                                 xts[di][:, sl], start=(di == 0),
                                 stop=(di == n_dch - 1))
            for di in range(n_dch):
                nc.tensor.matmul(ps_bot[:], w_tiles[di][:, col0 + 128:col0 + 256],
                                 xts[di][:, sl], start=(di == 0),
                                 stop=(di == n_dch - 1))
            return ps_top, ps_bot

        def emit_rope(job):
            top_dst, bot_dst, m, ps_top, ps_bot = job
            sl = slice(m * TBLK, (m + 1) * TBLK)
            c_sl, s_sl = cos_sb[:, sl], sin_sb[:, sl]
            t1 = poolT.tile([128, TBLK], F32, name="rt1", tag="tmp")
            t2 = poolT.tile([128, TBLK], F32, name="rt2", tag="tmp")
            nc.vector.tensor_mul(t1[:], ps_top[:], c_sl)
            nc.vector.tensor_mul(t2[:], ps_bot[:], s_sl)
            nc.vector.tensor_sub(top_dst[:, sl], t1[:], t2[:])
            t3 = poolT.tile([128, TBLK], F32, name="rt3", tag="tmp")
            t4 = poolT.tile([128, TBLK], F32, name="rt4", tag="tmp")
            nc.vector.tensor_mul(t3[:], ps_bot[:], c_sl)
            nc.vector.tensor_mul(t4[:], ps_top[:], s_sl)
            nc.vector.tensor_add(bot_dst[:, sl], t3[:], t4[:])

        in_bounce = poolD.tile([t, d], BF, name="in_bounce")
        out_bounces = {}
        rs_done = []
        last_dma = [None]

        def emit_attention(t0, tw, h, mid_hook=None, pool=None, enc=None,
                           small=False):
            pool = pool or poolB
            enc = enc or encT
            bx, bf_, brc = (3, 2, 2) if small else (8, 8, 4)
            t_sl = slice(t0, t0 + tw)
            n_chunks = (t0 + tw) // SCH if causal else n_sch
            ps_e0 = poolPS.tile([128, tw], F32, name="ps_e0", tag="enc", bufs=2)
            ps_e1 = poolPS.tile([128, tw], F32, name="ps_e1", tag="enc", bufs=2)
            ps_ds = poolPS.tile([1, tw], F32, name="ps_ds", tag="aux", bufs=2)

            def emit_logits_exp(j):
                s_sl = slice(j * SCH, (j + 1) * SCH)
                ps_l = poolPS.tile([128, tw], F32, name="ps_l", tag="qk", bufs=2)
                nc.tensor.matmul(ps_l[:], ktop[:, s_sl], qtop[h][:, t_sl],
                                 start=True, stop=False)
                nc.tensor.matmul(ps_l[:], kbot[:, s_sl], qbot[h][:, t_sl],
                                 start=False, stop=True)
                ex = pool.tile([128, TBLK], BF, name="ex", tag="ex", bufs=bx)
                nc.scalar.activation(ex[:, :tw], ps_l[:],
                                     mybir.ActivationFunctionType.Exp)
                if causal:
                    if j >= t0 // SCH:
                        i = j - t0 // SCH
                        nc.vector.tensor_mul(
                            ex[:, :tw], ex[:, :tw],
                            cm_sb[:, i * TBLK:i * TBLK + tw])
                elif masked:
                    gm = poolG.tile([128, TBLK], BF, name="gm", tag="gm")
                    nc.sync.dma_start(gm[:, :tw], gm_e.ap()[s_sl, t_sl])
                    nc.vector.tensor_mul(ex[:, :tw], ex[:, :tw], gm[:, :tw])
                return ex

            ex_q = [emit_logits_exp(jj) for jj in range(min(2, n_chunks))]
            hooks = dict(mid_hook or {})

            def run_hooks(j):
                for k in sorted(hooks):
                    if j is None or k <= j:
                        hooks.pop(k)()

            for j in range(n_chunks):
                run_hooks(j)
                ex = ex_q.pop(0)
                if j + 2 < n_chunks:
                    ex_q.append(emit_logits_exp(j + 2))
                last = j == n_chunks - 1
                nc.tensor.matmul(ps_e0[:], v_sb[j][:, 0:128], ex[:, :tw],
                                 start=(j == 0), stop=last)
                nc.tensor.matmul(ps_e1[:], v_sb[j][:, 128:256], ex[:, :tw],
                                 start=(j == 0), stop=last)
                nc.tensor.matmul(ps_ds[:], ones_col[:], ex[:, :tw],
                                 start=(j == 0), stop=last)
            run_hooks(None)

            ef0 = pool.tile([128, TBLK], F32, name="ef0", tag="ef", bufs=bf_)
            ef1 = pool.tile([128, TBLK], F32, name="ef1", tag="ef", bufs=bf_)
            nc.vector.tensor_copy(ef0[:, :tw], ps_e0[:])
            nc.vector.tensor_copy(ef1[:, :tw], ps_e1[:])
            rrow = pool.tile([1, TBLK], F32, name="rrow", tag="rrow", bufs=brc)
            nc.vector.reciprocal_approx_fast(rrow[:, :tw], ps_ds[:])
            rbc = pool.tile([128, TBLK], F32, name="rbc", tag="rbc", bufs=brc)
            nc.gpsimd.partition_broadcast(rbc[:, :tw], rrow[:, :tw])
            return (ef0, ef1, rbc, t_sl, tw, h, enc)

        def emit_norm(job):
            ef0, ef1, rbc, t_sl, tw, h, enc = job
            nc.vector.tensor_mul(enc[2 * h][:, t_sl], ef0[:, :tw], rbc[:, :tw])
            nc.vector.tensor_mul(enc[2 * h + 1][:, t_sl], ef1[:, :tw], rbc[:, :tw])

        def emit_wo_rs(t0, tw, pool=None, enc=None, small=False):
            pool = pool or poolB
            enc = enc or encT
            for tt in range(tw // TT):
                r_sl = slice(t0 + tt * TT, t0 + (tt + 1) * TT)
                for k_db in range(n_dblk):
                    d_sl = slice(k_db * TBLK, (k_db + 1) * TBLK)
                    ps_o = poolPS.tile([128, TBLK], F32, name="ps_o", tag="wo", bufs=2)
                    for k in range(4):
                        nc.tensor.matmul(ps_o[:], enc[k][:, r_sl],
                                         wos[k][:, d_sl], start=(k == 0),
                                         stop=(k == 3))
                    ostg = pool.tile([128, TBLK], BF, name="ostg", tag="ostg",
                                     bufs=3 if small else 6)
                    nc.vector.tensor_copy(ostg[:], ps_o[:])
                    last_dma[0] = nc.sync.dma_start(in_bounce[r_sl, d_sl], ostg[:])
            ob = poolD.tile([tw // 4, d], BF, name=f"out_b{t0}")
            out_bounces[t0] = ob
            nc.gpsimd.collective_compute(
                "ReduceScatter", mybir.AluOpType.add,
                replica_groups=GROUPS,
                ins=[in_bounce[t0:t0 + tw, :].opt()],
                outs=[ob.opt()])
            rs_done.append((t0, tw))

        early0 = causal and n_tblk >= 3
        if early0:
            enc0 = [poolB0.tile([128, TBLK], BF, name=f"enc0_{k}")
                    for k in range(4)]
        regions = rs_regions(t)
        pending = None
        for m in range(n_tblk):
            for j in range(4 * m, 4 * m + 4):
                ps_v = poolPS.tile([128, H], F32, name="ps_v",
                                   tag="wo" if j % 2 == 0 else "aux", bufs=2)
                for di in range(n_dch):
                    nc.tensor.matmul(ps_v[:], xts[di][:, j * SCH:(j + 1) * SCH],
                                     wvs[di][:], start=(di == 0),
                                     stop=(di == n_dch - 1))
                nc.vector.tensor_copy(v_sb[j][:], ps_v[:])
            for (top_dst, bot_dst, w_tiles, col0) in (
                    (ktop, kbot, wks, 0),
                    (qtop[0], qbot[0], wqs, 0),
                    (qtop[1], qbot[1], wqs, H)):
                ps_top, ps_bot = emit_proj(w_tiles, col0, m)
                if pending is not None:
                    emit_rope(pending)
                pending = (top_dst, bot_dst, m, ps_top, ps_bot)
            if early0 and m == 1:
                emit_rope(pending)
                pending = None
                e0j0 = emit_attention(0, TBLK, 0, pool=poolB0, enc=enc0,
                                      small=True)
                emit_norm(e0j0)
                e0j1 = emit_attention(0, TBLK, 1, pool=poolB0, enc=enc0,
                                      small=True)
                emit_norm(e0j1)
            if early0 and m == 2:
                emit_wo_rs(0, TBLK, pool=poolB0, enc=enc0, small=True)
        if pending is not None:
            emit_rope(pending)

        poolW.release()
        poolB = tc.alloc_tile_pool(name="phaseB", bufs=1)
        if masked:
            poolG = tc.alloc_tile_pool(name="gmask", bufs=4)
        encT = [poolB.tile([128, t], BF, name=f"enc{k}") for k in range(4)]

        if early0:
            regions = regions[1:]
        wo_pending = None

        def norm_pending():
            emit_norm(wo_pending[2][0])
            emit_norm(wo_pending[2][1])

        def flush_pending():
            emit_wo_rs(wo_pending[0], wo_pending[1])

        for (t0, tw) in regions:
            hooks = ({1: norm_pending, 5: flush_pending}
                     if wo_pending is not None else None)
            j0 = emit_attention(t0, tw, 0, mid_hook=hooks)
            j1 = emit_attention(t0, tw, 1)
            wo_pending = (t0, tw, [j0, j1])
        norm_pending()
        flush_pending()
        tc.no_sync_barrier()
        for (t0, tw) in rs_done:
            nc.gpsimd.dma_start(
                out_e.ap()[t0 // 4:(t0 + tw) // 4, :], out_bounces[t0][:])

        if masked:
            poolG.release()
        poolB.release()
        poolB0.release()
        poolD.release()
        poolPS.release()
        poolT.release()
        poolP.release()

    nc.compile()
    return nc


_NC_CACHE = {}


def _get_nc(variant, t=T, d=D):
    key = (variant, t, d)
    if key not in _NC_CACHE:
        if variant == "causal":
            _NC_CACHE[key] = build_fast()
        else:
            _NC_CACHE[key] = build(variant, t, d)
    return _NC_CACHE[key]


def _rope_tables(pos):
    half = H // 2
    freq_exp = (2.0 / H) * np.arange(half, dtype=np.float32)
    timescale = (MAX_WAVELENGTH ** freq_exp).astype(np.float32)
    radians = pos[None, :].astype(np.float32) / timescale[:, None]
    return np.cos(radians).astype(np.float32), np.sin(radians).astype(np.float32)


def _causal_tiles():
    ds = np.arange(SCH)[:, None]
    dt = np.arange(TBLK)[None, :]
    tiles = [(dt >= ds + SCH * i).astype(np.float32) for i in range(4)]
    return np.concatenate(tiles, axis=1).astype(ml_dtypes.bfloat16)


def _prep_in_maps(x, positions, attn_mask, wq, wkv, wo, variant):
    if variant == "causal":
        return _prep_fast_in_maps(x, positions, wq, wkv, wo)
    causal = False
    bf = ml_dtypes.bfloat16
    scale = np.float32(H) ** np.float32(-0.5)
    wq_s = (np.asarray(wq, np.float32) * scale)
    wk = np.asarray(wkv[0, 0], np.float32).astype(bf)
    wv = np.asarray(wkv[1, 0], np.float32).astype(bf)

    in_maps = []
    for c in range(N_CORES):
        b, r = divmod(c, 4)
        h0, h1 = 2 * r, 2 * r + 1
        xT = np.ascontiguousarray(np.asarray(x[b], np.float32).T).astype(bf)
        wq_c = np.ascontiguousarray(
            np.concatenate([wq_s[h0], wq_s[h1]], axis=1)).astype(bf)
        wo_c = np.ascontiguousarray(
            np.concatenate([np.asarray(wo[h0], np.float32),
                            np.asarray(wo[h1], np.float32)], axis=0)).astype(bf)
        cosT, sinT = _rope_tables(np.asarray(positions[b], np.float32))
        m = {"xT": xT, "wq": wq_c, "wk": wk, "wv": wv, "wo": wo_c,
             "cosT": cosT, "sinT": sinT}
        if variant == "masked":
            m["gmask"] = np.ascontiguousarray(
                np.asarray(attn_mask[b, 0], np.float32).T).astype(bf)
        in_maps.append(m)
    return in_maps


def kernel(x, positions, attn_mask, wq, wkv, wo):
    x = np.asarray(x)
    positions = np.asarray(positions)
    attn_mask = np.asarray(attn_mask)
    wq, wkv, wo = np.asarray(wq), np.asarray(wkv), np.asarray(wo)

    tril = np.tril(np.ones((T, T), bool))
    if all(np.array_equal(attn_mask[b, 0], tril) for b in range(B)):
        variant = "causal"
    elif attn_mask.all():
        variant = "dense"
    else:
        variant = "masked"

    nc = _get_nc(variant)
    in_maps = _prep_in_maps(x, positions, attn_mask, wq, wkv, wo, variant)
    res = bass_utils.run_bass_kernel_spmd(nc, in_maps,
                                          core_ids=list(range(N_CORES)))

    if variant == "causal":
        return _unshard_fast(res.results)

    out = np.empty((B, T, D), np.float32)
    for c in range(N_CORES):
        b, r = divmod(c, 4)
        shard = np.asarray(res.results[c]["out"], dtype=np.float32)
        for (t0, tw) in rs_regions(T):
            rows = tw // 4
            out[b, t0 + r * rows:t0 + (r + 1) * rows, :] = \
                shard[t0 // 4:t0 // 4 + rows, :]
    return out


# revision 28
# speedup vs baseline: 1.0556x; 1.0556x over previous
"""Distributed Trainium2 kernel for GQA attention (B=2, T=2048, D=2048, N=8
query heads, K=1 KV head, H=256) on 8 NeuronCores.

Sharding (causal fast path): 2 (batch) x 4 (sequence) mesh with NO collectives.
Core c = 4*b + r handles batch b and query chunks {r, 4+r, 8+r, 12+r} (128
tokens each; strided assignment balances causal work exactly). Each core
computes K/V over the full sequence (replicated within the batch group), all 8
heads of attention for its 512 query tokens, and the full out-projection for
those rows -- the head sum is local, so each core DMAs out its own 512 rows.

The graph is identical on all 8 cores (single NEFF). Per-core differences are
pure data:
  - x columns are rolled by -128*r inside each 512 block so the owned query
    chunks sit at physical positions {0, 512, 1024, 1536}; K/V inherit the
    rolled order and the causal structure only depends on the group index
    l//4, which the roll preserves.
  - the partially/fully masked "band" tiles (key chunk in the same group-of-4
    as the query block) are 0/1 data tiles: for physical band slot k',
    logical k = (k'+r)%4: k<r all-ones, k==r diagonal, k>r all-zeros.

Device layout (transposed attention, all bf16):
  xT [D, T] rolled; qT per head [256, 512-owned]; kT [256, T]; v [T, 256]
  logitsT [key-chunk 128, q-suffix w] where w = 512 - 128*(chunk//4) -- the
  suffix structure computes exactly the causally-needed chunks (+ the band
  slack), uniformly across cores.
  exp via ScalarE (bias -2 keeps exp in range without a max pass), dsum via
  ones-column matmul, reciprocal + partition broadcast, normalize into enc,
  out[tok 128, D-blk 512] = sum over 16 enc chunks @ wo.

The dense/masked variants keep the legacy 2x4 head-parallel kernel with the
chunked ReduceScatter (below).
"""

import numpy as np
import ml_dtypes

import concourse.bass as bass
import concourse.bacc as bacc
import concourse.mybir as mybir
import concourse.tile as tile
from concourse import bass_utils

BF = mybir.dt.bfloat16
F32 = mybir.dt.float32

B, T, D, N, KVH, H = 2, 2048, 2048, 8, 1, 256
MAX_WAVELENGTH = 10000
TBLK = 512    # T block (matmul moving free dim / PSUM bank)
SCH = 128     # S chunk (key chunk, PSUM partition dim)
TT = 128      # T tile (out-projection partition dim)
RS_ROWS = 256  # rows per ReduceScatter chunk (legacy)
GROUPS = [[0, 1, 2, 3], [4, 5, 6, 7]]
N_CORES = 8
NDCH = D // 128   # 16 D chunks
NQB = 4           # owned query blocks per core
QW = NQB * SCH    # 512 owned query columns


# --------------------------------------------------------------------------
# causal fast path: sequence-parallel, no collectives
# --------------------------------------------------------------------------

def build_fast():
    nc = bacc.Bacc("TRN2", target_bir_lowering=False, debug=False,
                   num_devices=N_CORES)

    x_e = nc.dram_tensor("xT", [D, T], BF, kind="ExternalInput")
    wq_e = nc.dram_tensor("wq", [D, N * H], BF, kind="ExternalInput")
    wk_e = nc.dram_tensor("wk", [D, H], BF, kind="ExternalInput")
    wv_e = nc.dram_tensor("wv", [D, H], BF, kind="ExternalInput")
    wo_e = nc.dram_tensor("wo", [N * H, D], BF, kind="ExternalInput")
    cosq_e = nc.dram_tensor("cosq", [H // 2, QW], BF, kind="ExternalInput")
    sinq_e = nc.dram_tensor("sinq", [H // 2, QW], BF, kind="ExternalInput")
    cosk_e = nc.dram_tensor("cosk", [H // 2, T], BF, kind="ExternalInput")
    sink_e = nc.dram_tensor("sink", [H // 2, T], BF, kind="ExternalInput")
    cm_e = nc.dram_tensor("cmask", [SCH, 4 * SCH], BF, kind="ExternalInput")
    out_e = nc.dram_tensor("out", [QW, D], BF, kind="ExternalOutput")

    with tile.TileContext(nc) as tc:
        poolP = tc.alloc_tile_pool(name="persist", bufs=1)
        poolT = tc.alloc_tile_pool(name="tmps", bufs=6)
        poolPS = tc.alloc_tile_pool(name="ps", bufs=1, space="PSUM")
        poolW = tc.alloc_tile_pool(name="w", bufs=1)

        # ---- input SBUF tiles -------------------------------------------
        x_sb = poolW.tile([128, NDCH * T], BF, name="x_sb")
        # wq holds 4 heads at a time; reloaded with heads 4-7 once heads 0-3
        # have projected
        wq_sb = poolW.tile([128, NDCH * 4 * H], BF, name="wq_sb")
        wk_sb = poolW.tile([128, NDCH * H], BF, name="wk_sb")
        wv_sb = poolW.tile([128, NDCH * H], BF, name="wv_sb")
        cosq_sb = poolW.tile([128, QW], BF, name="cosq_sb")
        sinq_sb = poolW.tile([128, QW], BF, name="sinq_sb")
        cosk_sb = poolW.tile([128, T], BF, name="cosk_sb")
        sink_sb = poolW.tile([128, T], BF, name="sink_sb")

        x3 = x_sb.rearrange("p (i t) -> p i t", i=NDCH)
        # [p, dchunk, block m, col] view: owned cols are [:, :, :, 0:128]
        x4 = x_sb.rearrange("p (i m c) -> p i m c", i=NDCH, m=NQB)
        wq3 = wq_sb.rearrange("p (i c) -> p i c", i=NDCH)
        wk3 = wk_sb.rearrange("p (i c) -> p i c", i=NDCH)
        wv3 = wv_sb.rearrange("p (i c) -> p i c", i=NDCH)

        x_r = x_e.ap().rearrange("(i p) t -> p i t", p=128)
        wq_r = wq_e.ap().rearrange("(i p) c -> p i c", p=128)

        def load_w(dst3, src, i0, i1):
            src_r = src.ap().rearrange("(i p) c -> p i c", p=128)
            nc.sync.dma_start(dst3[:, i0:i1, :], src_r[:, i0:i1, :])

        hd = NDCH // 2

        def load_x_block(m):
            sl = slice(m * TBLK, (m + 1) * TBLK)
            nc.sync.dma_start(x3[:, 0:hd, sl], x_r[:, 0:hd, sl])
            nc.sync.dma_start(x3[:, hd:NDCH, sl], x_r[:, hd:NDCH, sl])

        # K proj of block 0 is the first consumer: its inputs go first
        load_w(wk3, wk_e, 0, hd)
        nc.sync.dma_start(x3[:, 0:hd, 0:TBLK], x_r[:, 0:hd, 0:TBLK])
        load_w(wk3, wk_e, hd, NDCH)
        nc.sync.dma_start(x3[:, hd:NDCH, 0:TBLK], x_r[:, hd:NDCH, 0:TBLK])
        load_w(wv3, wv_e, 0, NDCH)
        nc.sync.dma_start(cosk_sb[:], cosk_e.ap()[:, :])
        nc.sync.dma_start(sink_sb[:], sink_e.ap()[:, :])
        nc.sync.dma_start(cosq_sb[:], cosq_e.ap()[:, :])
        nc.sync.dma_start(sinq_sb[:], sinq_e.ap()[:, :])
        cm_sb = poolP.tile([SCH, 4 * SCH], BF, name="cm_sb")
        nc.sync.dma_start(cm_sb[:], cm_e.ap()[:, :])
        load_x_block(1)
        nc.sync.dma_start(wq3[:, :, :], wq_r[:, :, 0:4 * H])
        load_x_block(2)
        load_x_block(3)

        ones_col = poolP.tile([128, 1], BF, name="ones_col")
        nc.vector.memset(ones_col[:], 1.0)
        # warm the gpsimd partition_broadcast library while DMAs stream so
        # the first real broadcast doesn't pay the ~12us LOAD_LIB on the
        # norm critical path
        warm1 = poolP.tile([1, 8], F32, name="warm1")
        warm2 = poolP.tile([128, 8], F32, name="warm2")
        nc.vector.memset(warm1[:], 1.0)
        nc.gpsimd.partition_broadcast(warm2[:], warm1[:])

        # persistent homes for attention-phase outputs so they don't alias
        # the phase-A weight pool (aliasing would stall head 0's norm until
        # every weight read completes)
        enc_sb = poolP.tile([128, NDCH * QW], BF, name="enc_sb")
        enc3 = enc_sb.rearrange("p (i c) -> p i c", i=NDCH)

        # ---- persistent activation tiles --------------------------------
        k_sb = poolP.tile([128, 2 * T], BF, name="k_sb")      # [top|bot]
        v_sb = poolP.tile([128, (T // SCH) * H], BF, name="v_sb")
        q_all = poolP.tile([128, N * 2 * QW], BF, name="q_all")
        v3 = v_sb.rearrange("p (j c) -> p j c", j=T // SCH)

        def qtop(h):
            return q_all[:, h * 2 * QW:h * 2 * QW + QW]

        def qbot(h):
            return q_all[:, h * 2 * QW + QW:(h + 1) * 2 * QW]

        # ---- phase A: projections + rope --------------------------------
        def emit_qproj(h):
            hh = h % 4
            ps_qt = poolPS.tile([128, QW], F32, name="ps_qt", tag="qk", bufs=3)
            ps_qb = poolPS.tile([128, QW], F32, name="ps_qb", tag="qk", bufs=3)
            for di in range(NDCH):
                nc.tensor.matmul(ps_qt[:], wq3[:, di, hh * H:hh * H + 128],
                                 x4[:, di, :, 0:128], start=(di == 0),
                                 stop=(di == NDCH - 1))
            for di in range(NDCH):
                nc.tensor.matmul(ps_qb[:], wq3[:, di, hh * H + 128:(hh + 1) * H],
                                 x4[:, di, :, 0:128], start=(di == 0),
                                 stop=(di == NDCH - 1))
            return ("q", h, ps_qt, ps_qb)

        def emit_kproj(bk):
            sl = slice(bk * TBLK, (bk + 1) * TBLK)
            ps_kt = poolPS.tile([128, TBLK], F32, name="ps_kt", tag="enc", bufs=3)
            ps_kb = poolPS.tile([128, TBLK], F32, name="ps_kb", tag="enc", bufs=3)
            for di in range(NDCH):
                nc.tensor.matmul(ps_kt[:], wk3[:, di, 0:128], x3[:, di, sl],
                                 start=(di == 0), stop=(di == NDCH - 1))
            for di in range(NDCH):
                nc.tensor.matmul(ps_kb[:], wk3[:, di, 128:256], x3[:, di, sl],
                                 start=(di == 0), stop=(di == NDCH - 1))
            return ("k", bk, ps_kt, ps_kb)

        def emit_vchunk(j):
            ps_v = poolPS.tile([128, H], F32, name="ps_v", tag="aux", bufs=2)
            for di in range(NDCH):
                nc.tensor.matmul(ps_v[:], x3[:, di, j * SCH:(j + 1) * SCH],
                                 wv3[:, di, :], start=(di == 0),
                                 stop=(di == NDCH - 1))
            nc.vector.tensor_copy(v3[:, j, :], ps_v[:])

        def emit_rope(job):
            kind, idx, ps_t, ps_b = job
            if kind == "q":
                c_sl, s_sl = cosq_sb[:, :], sinq_sb[:, :]
                top_dst, bot_dst = qtop(idx), qbot(idx)
            else:
                sl = slice(idx * TBLK, (idx + 1) * TBLK)
                c_sl, s_sl = cosk_sb[:, sl], sink_sb[:, sl]
                top_dst, bot_dst = k_sb[:, sl], k_sb[:, T + idx * TBLK:
                                                     T + (idx + 1) * TBLK]
            t1 = poolT.tile([128, TBLK], F32, name="rt1", tag="tmp")
            t4 = poolT.tile([128, TBLK], F32, name="rt4", tag="tmp")
            nc.vector.tensor_mul(t1[:], ps_t[:], c_sl)
            nc.vector.tensor_mul(t4[:], ps_t[:], s_sl)
            t2 = poolT.tile([128, TBLK], F32, name="rt2", tag="tmp")
            t3 = poolT.tile([128, TBLK], F32, name="rt3", tag="tmp")
            nc.vector.tensor_mul(t2[:], ps_b[:], s_sl)
            nc.vector.tensor_mul(t3[:], ps_b[:], c_sl)
            nc.vector.tensor_sub(top_dst, t1[:], t2[:])
            nc.vector.tensor_add(bot_dst, t3[:], t4[:])

        # ---- attention logits/exp (pre-emitted for head 0 inside the
        # projection phase so the exp pipeline is primed when PV starts) ---
        # group g covers physical key chunks 4g..4g+3, all with query-suffix
        # width w = 512-128*g starting at qo = 128*g.
        def emit_le(h, g):
            """logits+exp for the 4 chunks of group g, then the band mask."""
            qo = SCH * g
            w = QW - qo
            ex = poolP.tile([128, 4 * TBLK], BF, name="ex", tag="ex", bufs=4)
            for ci in range(4):
                l = 4 * g + ci
                ps_l = poolPS.tile([128, TBLK], F32, name="ps_l", tag="qk",
                                   bufs=3)
                nc.tensor.matmul(ps_l[:, :w], k_sb[:, l * SCH:(l + 1) * SCH],
                                 qtop(h)[:, qo:], start=True, stop=False)
                nc.tensor.matmul(ps_l[:, :w],
                                 k_sb[:, T + l * SCH:T + (l + 1) * SCH],
                                 qbot(h)[:, qo:], start=False, stop=True)
                nc.scalar.activation(ex[:, ci * TBLK + qo:(ci + 1) * TBLK],
                                     ps_l[:, :w],
                                     mybir.ActivationFunctionType.Exp)
            ex4 = ex.rearrange("p (c x) -> p c x", c=4)
            nc.vector.tensor_mul(ex4[:, :, qo:qo + SCH], ex4[:, :, qo:qo + SCH],
                                 cm_sb.rearrange("p (c x) -> p c x", c=4))
            return (g, qo, w, ex)

        def emit_pv(h, job, ps_e0, ps_e1, ps_ds):
            g, qo, w, ex = job
            for ci in range(4):
                l = 4 * g + ci
                e_sl = slice(ci * TBLK + qo, (ci + 1) * TBLK)
                start = l == 0
                stop = l == 15
                nc.tensor.matmul(ps_e0[:, qo:], v3[:, l, 0:128], ex[:, e_sl],
                                 start=start, stop=stop)
                nc.tensor.matmul(ps_e1[:, qo:], v3[:, l, 128:256], ex[:, e_sl],
                                 start=start, stop=stop)
                nc.tensor.matmul(ps_ds[:, qo:], ones_col[:], ex[:, e_sl],
                                 start=start, stop=stop)

        # ---- phase A execution: KV blocks + Q heads with pending-rope
        # pipelining; head 0's logits/exp groups are pre-emitted as soon as
        # their K blocks are roped (le hooks), priming the attention phase.
        pending = None
        h0_jobs = []

        def run_item(item):
            nonlocal pending
            kind, idx = item
            if kind == "le":
                h0_jobs.append(emit_le(0, idx))
                return
            if kind == "wq2":
                nc.sync.dma_start(wq3[:, :, :], wq_r[:, :, 4 * H:8 * H])
                return
            job = emit_qproj(idx) if kind == "q" else emit_kproj(idx)
            if pending is not None:
                emit_rope(pending)
            pending = job
            if kind == "k":
                for j in range(4 * idx, 4 * idx + 4):
                    emit_vchunk(j)

        seq = [("k", 0), ("k", 1), ("q", 0), ("q", 1), ("le", 0), ("le", 1),
               ("k", 2), ("q", 2), ("le", 2), ("q", 3), ("wq2", 0), ("k", 3),
               ("q", 4), ("le", 3), ("q", 5), ("q", 6), ("q", 7)]
        for item in seq:
            run_item(item)
        emit_rope(pending)

        poolW.release()
        poolB = tc.alloc_tile_pool(name="phaseB", bufs=1)

        wo_sb = poolB.tile([128, NDCH * D], BF, name="wo_sb")
        wo3 = wo_sb.rearrange("p (i c) -> p i c", i=NDCH)
        wo_r = wo_e.ap().rearrange("(i p) c -> p i c", p=128)
        for s in range(4):
            i0, i1 = s * (NDCH // 4), (s + 1) * (NDCH // 4)
            nc.sync.dma_start(wo3[:, i0:i1, :], wo_r[:, i0:i1, :])

        # ---- phase B: attention per head --------------------------------
        for h in range(N):
            ps_e0 = poolPS.tile([128, QW], F32, name="ps_e0", tag="enc", bufs=3)
            ps_e1 = poolPS.tile([128, QW], F32, name="ps_e1", tag="enc", bufs=3)
            ps_ds = poolPS.tile([1, QW], F32, name="ps_ds", tag="aux", bufs=2)
            if h == 0:
                exq = h0_jobs
                next_g = 4
            else:
                exq = [emit_le(h, 0)]
                next_g = 1
            for g in range(4):
                if next_g < 4:
                    exq.append(emit_le(h, next_g))
                    next_g += 1
                emit_pv(h, exq.pop(0), ps_e0, ps_e1, ps_ds)
            rrow = poolP.tile([1, QW], F32, name="rrow", tag="rrow", bufs=2)
            nc.vector.reciprocal_approx_fast(rrow[:], ps_ds[:])
            rbc = poolP.tile([128, QW], F32, name="rbc", tag="rbc", bufs=2)
            nc.gpsimd.partition_broadcast(rbc[:], rrow[:])
            nc.vector.tensor_mul(enc3[:, 2 * h, :], ps_e0[:], rbc[:])
            nc.vector.tensor_mul(enc3[:, 2 * h + 1, :], ps_e1[:], rbc[:])

        # ---- phase C: out-projection ------------------------------------
        for tt in range(NQB):
            t_sl = slice(tt * TT, (tt + 1) * TT)
            for db in range(D // TBLK):
                d_sl = slice(db * TBLK, (db + 1) * TBLK)
                ps_o = poolPS.tile([128, TBLK], F32, name="ps_o", tag="qk",
                                   bufs=3)
                for k in range(NDCH):
                    nc.tensor.matmul(ps_o[:], enc3[:, k, t_sl], wo3[:, k, d_sl],
                                     start=(k == 0), stop=(k == NDCH - 1))
                ostg = poolB.tile([128, TBLK], BF, name="ostg", tag="ostg",
                                  bufs=4)
                nc.vector.tensor_copy(ostg[:], ps_o[:])
                nc.sync.dma_start(out_e.ap()[t_sl, d_sl], ostg[:])

        poolB.release()
        poolPS.release()
        poolT.release()
        poolP.release()

    nc.compile()
    return nc


def _rope_tables_cols(pos):
    """pos [ncols] f32 -> cos, sin [H/2, ncols] f32."""
    half = H // 2
    freq_exp = (2.0 / H) * np.arange(half, dtype=np.float32)
    timescale = (MAX_WAVELENGTH ** freq_exp).astype(np.float32)
    radians = pos[None, :].astype(np.float32) / timescale[:, None]
    return np.cos(radians), np.sin(radians)


def _prep_fast_in_maps(x, positions, wq, wkv, wo):
    bf = ml_dtypes.bfloat16
    scale = np.float32(H) ** np.float32(-0.5)
    wq_s = (np.asarray(wq, np.float32) * scale)
    # head-major columns [D, N*H]
    wq_cat = np.ascontiguousarray(
        np.concatenate([wq_s[h] for h in range(N)], axis=1)).astype(bf)
    wk = np.asarray(wkv[0, 0], np.float32).astype(bf)
    wv = np.asarray(wkv[1, 0], np.float32).astype(bf)
    wo_cat = np.ascontiguousarray(
        np.concatenate([np.asarray(wo[h], np.float32) for h in range(N)],
                       axis=0)).astype(bf)

    ds = np.arange(SCH)[:, None]
    dt_ = np.arange(SCH)[None, :]
    diag = (dt_ >= ds).astype(np.float32)
    ones = np.ones((SCH, SCH), np.float32)
    zeros = np.zeros((SCH, SCH), np.float32)

    in_maps = []
    for c in range(N_CORES):
        b, r = divmod(c, 4)
        # physical column order: roll by -128*r inside each 512 block
        idx = np.concatenate([
            m * TBLK + (np.arange(TBLK) + SCH * r) % TBLK
            for m in range(NQB)])
        xb = np.asarray(x[b], np.float32)          # [T, D]
        xT = np.ascontiguousarray(xb[idx].T).astype(bf)   # [D, T] rolled
        pos = np.asarray(positions[b], np.float32)
        pos_phys = pos[idx]
        cosk, sink = _rope_tables_cols(pos_phys)
        own = idx.reshape(NQB, TBLK)[:, 0:SCH].reshape(-1)
        cosq, sinq = _rope_tables_cols(pos[own])
        # band tiles by physical slot k': logical k = (k'+r)%4
        tiles = []
        for kp in range(4):
            k = (kp + r) % 4
            tiles.append(ones if k < r else (diag if k == r else zeros))
        cm = np.concatenate(tiles, axis=1).astype(bf)
        in_maps.append({
            "xT": xT, "wq": wq_cat, "wk": wk, "wv": wv, "wo": wo_cat,
            "cosq": cosq.astype(bf), "sinq": sinq.astype(bf),
            "cosk": cosk.astype(bf), "sink": sink.astype(bf), "cmask": cm,
        })
    return in_maps


def _unshard_fast(results):
    out = np.empty((B, T, D), np.float32)
    for c in range(N_CORES):
        b, r = divmod(c, 4)
        shard = np.asarray(results[c]["out"], dtype=np.float32)
        for i in range(NQB):
            t0 = (4 * i + r) * SCH
            out[b, t0:t0 + SCH, :] = shard[i * SCH:(i + 1) * SCH, :]
    return out


# --------------------------------------------------------------------------
# legacy path (dense / masked variants): 2 (batch) x 4 (head-pair) mesh with
# chunked ReduceScatter over the out-projection partial sums.
# --------------------------------------------------------------------------

def rs_regions(t):
    n_tblk = t // TBLK
    regions = [(m * TBLK, TBLK) for m in range(n_tblk - 1)]
    last0 = (n_tblk - 1) * TBLK
    return regions + [(last0, TBLK // 2), (last0 + TBLK // 2, TBLK // 2)]


def build(variant="causal", t=T, d=D):
    """Legacy SPMD graph (identical on all 8 cores)."""
    causal = variant == "causal"
    masked = variant == "masked"
    n_tblk = t // TBLK
    n_dch = d // 128
    n_dblk = d // TBLK
    n_sch = t // SCH
    n_rs = t // RS_ROWS

    nc = bacc.Bacc("TRN2", target_bir_lowering=False, debug=False,
                   num_devices=N_CORES)

    xT_e = nc.dram_tensor("xT", [d, t], BF, kind="ExternalInput")
    wq_e = nc.dram_tensor("wq", [d, 2 * H], BF, kind="ExternalInput")
    wk_e = nc.dram_tensor("wk", [d, H], BF, kind="ExternalInput")
    wv_e = nc.dram_tensor("wv", [d, H], BF, kind="ExternalInput")
    wo_e = nc.dram_tensor("wo", [2 * H, d], BF, kind="ExternalInput")
    cos_e = nc.dram_tensor("cosT", [H // 2, t], F32, kind="ExternalInput")
    sin_e = nc.dram_tensor("sinT", [H // 2, t], F32, kind="ExternalInput")
    if causal:
        cm_e = nc.dram_tensor("cmask", [SCH, 4 * TBLK], BF, kind="ExternalInput")
    if masked:
        gm_e = nc.dram_tensor("gmask", [t, t], BF, kind="ExternalInput")
    out_e = nc.dram_tensor("out", [t // 4, d], BF, kind="ExternalOutput")

    with tile.TileContext(nc) as tc:
        poolP = tc.alloc_tile_pool(name="persist", bufs=1)
        poolT = tc.alloc_tile_pool(name="tmps", bufs=4)
        poolPS = tc.alloc_tile_pool(name="ps", bufs=1, space="PSUM")
        poolD = tc.alloc_tile_pool(name="dram", bufs=1, space="DRAM")
        poolB0 = tc.alloc_tile_pool(name="region0", bufs=1)
        poolW = tc.alloc_tile_pool(name="w", bufs=1)

        x_sb = poolW.tile([128, n_dch * t], BF, name="x_sb")
        wq_sb = poolW.tile([128, n_dch * 2 * H], BF, name="wq_sb")
        wk_sb = poolW.tile([128, n_dch * H], BF, name="wk_sb")
        wv_sb = poolW.tile([128, n_dch * H], BF, name="wv_sb")
        xts = [x_sb[:, i * t:(i + 1) * t] for i in range(n_dch)]
        wqs = [wq_sb[:, i * 2 * H:(i + 1) * 2 * H] for i in range(n_dch)]
        wks = [wk_sb[:, i * H:(i + 1) * H] for i in range(n_dch)]
        wvs = [wv_sb[:, i * H:(i + 1) * H] for i in range(n_dch)]
        wos = [poolP.tile([128, d], BF, name=f"wot{k}") for k in range(4)]
        cos_sb = poolP.tile([128, t], F32, name="cos_sb")
        sin_sb = poolP.tile([128, t], F32, name="sin_sb")

        xT_r = xT_e.ap().rearrange("(i p) t -> p i t", p=128)
        x_sb3 = x_sb.rearrange("p (i t) -> p i t", i=n_dch)

        def load_x_cols(c0, c1, n_split=2):
            step = n_dch // n_split
            for s in range(n_split):
                i0, i1 = s * step, (s + 1) * step
                nc.sync.dma_start(x_sb3[:, i0:i1, c0:c1], xT_r[:, i0:i1, c0:c1])

        def load_w(dst, src, cols, n_split=2):
            src_r = src.ap().rearrange("(i p) c -> p i c", p=128)
            dst_r = dst.rearrange("p (i c) -> p i c", i=n_dch)
            step = n_dch // n_split
            for s in range(n_split):
                i0, i1 = s * step, (s + 1) * step
                nc.sync.dma_start(dst_r[:, i0:i1, :], src_r[:, i0:i1, :])

        load_w(wv_sb, wv_e, H)
        load_x_cols(0, TBLK, n_split=4)
        nc.sync.dma_start(cos_sb[:], cos_e.ap()[:, :])
        nc.sync.dma_start(sin_sb[:], sin_e.ap()[:, :])
        load_w(wk_sb, wk_e, H)
        load_w(wq_sb, wq_e, 2 * H)
        if causal:
            cm_sb = poolP.tile([SCH, 4 * TBLK], BF, name="cm_sb")
            nc.sync.dma_start(cm_sb[:], cm_e.ap()[:, :])
        if n_tblk > 1:
            load_x_cols(TBLK, t, n_split=4)
        for k in range(4):
            nc.sync.dma_start(wos[k][:], wo_e.ap()[128 * k:128 * (k + 1), :])

        ones_col = poolP.tile([128, 1], BF, name="ones_col")
        nc.vector.memset(ones_col[:], 1.0)

        v_sb = [poolP.tile([128, H], BF, name=f"v{j}") for j in range(n_sch)]
        ktop = poolP.tile([128, t], BF, name="ktop")
        kbot = poolP.tile([128, t], BF, name="kbot")
        qtop = [poolP.tile([128, t], BF, name=f"qtop{h}") for h in range(2)]
        qbot = [poolP.tile([128, t], BF, name=f"qbot{h}") for h in range(2)]

        def emit_proj(w_tiles, col0, m):
            sl = slice(m * TBLK, (m + 1) * TBLK)
            ps_top = poolPS.tile([128, TBLK], F32, name="ps_top", tag="qk", bufs=2)
            ps_bot = poolPS.tile([128, TBLK], F32, name="ps_bot", tag="enc", bufs=2)
            for di in range(n_dch):
                nc.tensor.matmul(ps_top[:], w_tiles[di][:, col0:col0 + 128],
                                 xts[di][:, sl], start=(di == 0),
                                 stop=(di == n_dch - 1))
            for di in range(n_dch):
                nc.tensor.matmul(ps_bot[:], w_tiles[di][:, col0 + 128:col0 + 256],
                                 xts[di][:, sl], start=(di == 0),
                                 stop=(di == n_dch - 1))
            return ps_top, ps_bot

        def emit_rope(job):
            top_dst, bot_dst, m, ps_top, ps_bot = job
            sl = slice(m * TBLK, (m + 1) * TBLK)
            c_sl, s_sl = cos_sb[:, sl], sin_sb[:, sl]
            t1 = poolT.tile([128, TBLK], F32, name="rt1", tag="tmp")
            t2 = poolT.tile([128, TBLK], F32, name="rt2", tag="tmp")
            nc.vector.tensor_mul(t1[:], ps_top[:], c_sl)
            nc.vector.tensor_mul(t2[:], ps_bot[:], s_sl)
            nc.vector.tensor_sub(top_dst[:, sl], t1[:], t2[:])
            t3 = poolT.tile([128, TBLK], F32, name="rt3", tag="tmp")
            t4 = poolT.tile([128, TBLK], F32, name="rt4", tag="tmp")
            nc.vector.tensor_mul(t3[:], ps_bot[:], c_sl)
            nc.vector.tensor_mul(t4[:], ps_top[:], s_sl)
            nc.vector.tensor_add(bot_dst[:, sl], t3[:], t4[:])

        in_bounce = poolD.tile([t, d], BF, name="in_bounce")
        out_bounces = {}
        rs_done = []
        last_dma = [None]

        def emit_attention(t0, tw, h, mid_hook=None, pool=None, enc=None,
                           small=False):
            pool = pool or poolB
            enc = enc or encT
            bx, bf_, brc = (3, 2, 2) if small else (8, 8, 4)
            t_sl = slice(t0, t0 + tw)
            n_chunks = (t0 + tw) // SCH if causal else n_sch
            ps_e0 = poolPS.tile([128, tw], F32, name="ps_e0", tag="enc", bufs=2)
            ps_e1 = poolPS.tile([128, tw], F32, name="ps_e1", tag="enc", bufs=2)
            ps_ds = poolPS.tile([1, tw], F32, name="ps_ds", tag="aux", bufs=2)

            def emit_logits_exp(j):
                s_sl = slice(j * SCH, (j + 1) * SCH)
                ps_l = poolPS.tile([128, tw], F32, name="ps_l", tag="qk", bufs=2)
                nc.tensor.matmul(ps_l[:], ktop[:, s_sl], qtop[h][:, t_sl],
                                 start=True, stop=False)
                nc.tensor.matmul(ps_l[:], kbot[:, s_sl], qbot[h][:, t_sl],
                                 start=False, stop=True)
                ex = pool.tile([128, TBLK], BF, name="ex", tag="ex", bufs=bx)
                nc.scalar.activation(ex[:, :tw], ps_l[:],
                                     mybir.ActivationFunctionType.Exp)
                if causal:
                    if j >= t0 // SCH:
                        i = j - t0 // SCH
                        nc.vector.tensor_mul(
                            ex[:, :tw], ex[:, :tw],
                            cm_sb[:, i * TBLK:i * TBLK + tw])
                elif masked:
                    gm = poolG.tile([128, TBLK], BF, name="gm", tag="gm")
                    nc.sync.dma_start(gm[:, :tw], gm_e.ap()[s_sl, t_sl])
                    nc.vector.tensor_mul(ex[:, :tw], ex[:, :tw], gm[:, :tw])
                return ex

            ex_q = [emit_logits_exp(jj) for jj in range(min(2, n_chunks))]
            hooks = dict(mid_hook or {})

            def run_hooks(j):
                for k in sorted(hooks):
                    if j is None or k <= j:
                        hooks.pop(k)()

            for j in range(n_chunks):
                run_hooks(j)
                ex = ex_q.pop(0)
                if j + 2 < n_chunks:
                    ex_q.append(emit_logits_exp(j + 2))
                last = j == n_chunks - 1
                nc.tensor.matmul(ps_e0[:], v_sb[j][:, 0:128], ex[:, :tw],
                                 start=(j == 0), stop=last)
                nc.tensor.matmul(ps_e1[:], v_sb[j][:, 128:256], ex[:, :tw],
                                 start=(j == 0), stop=last)
                nc.tensor.matmul(ps_ds[:], ones_col[:], ex[:, :tw],
                                 start=(j == 0), stop=last)
            run_hooks(None)

            ef0 = pool.tile([128, TBLK], F32, name="ef0", tag="ef", bufs=bf_)
            ef1 = pool.tile([128, TBLK], F32, name="ef1", tag="ef", bufs=bf_)
            nc.vector.tensor_copy(ef0[:, :tw], ps_e0[:])
            nc.vector.tensor_copy(ef1[:, :tw], ps_e1[:])
            rrow = pool.tile([1, TBLK], F32, name="rrow", tag="rrow", bufs=brc)
            nc.vector.reciprocal_approx_fast(rrow[:, :tw], ps_ds[:])
            rbc = pool.tile([128, TBLK], F32, name="rbc", tag="rbc", bufs=brc)
            nc.gpsimd.partition_broadcast(rbc[:, :tw], rrow[:, :tw])
            return (ef0, ef1, rbc, t_sl, tw, h, enc)

        def emit_norm(job):
            ef0, ef1, rbc, t_sl, tw, h, enc = job
            nc.vector.tensor_mul(enc[2 * h][:, t_sl], ef0[:, :tw], rbc[:, :tw])
            nc.vector.tensor_mul(enc[2 * h + 1][:, t_sl], ef1[:, :tw], rbc[:, :tw])

        def emit_wo_rs(t0, tw, pool=None, enc=None, small=False):
            pool = pool or poolB
            enc = enc or encT
            for tt in range(tw // TT):
                r_sl = slice(t0 + tt * TT, t0 + (tt + 1) * TT)
                for k_db in range(n_dblk):
                    d_sl = slice(k_db * TBLK, (k_db + 1) * TBLK)
                    ps_o = poolPS.tile([128, TBLK], F32, name="ps_o", tag="wo", bufs=2)
                    for k in range(4):
                        nc.tensor.matmul(ps_o[:], enc[k][:, r_sl],
                                         wos[k][:, d_sl], start=(k == 0),
                                         stop=(k == 3))
                    ostg = pool.tile([128, TBLK], BF, name="ostg", tag="ostg",
                                     bufs=3 if small else 6)
                    nc.vector.tensor_copy(ostg[:], ps_o[:])
                    last_dma[0] = nc.sync.dma_start(in_bounce[r_sl, d_sl], ostg[:])
            ob = poolD.tile([tw // 4, d], BF, name=f"out_b{t0}")
            out_bounces[t0] = ob
            nc.gpsimd.collective_compute(
                "ReduceScatter", mybir.AluOpType.add,
                replica_groups=GROUPS,
                ins=[in_bounce[t0:t0 + tw, :].opt()],
                outs=[ob.opt()])
            rs_done.append((t0, tw))

        early0 = causal and n_tblk >= 3
        if early0:
            enc0 = [poolB0.tile([128, TBLK], BF, name=f"enc0_{k}")
                    for k in range(4)]
        regions = rs_regions(t)
        pending = None
        for m in range(n_tblk):
            for j in range(4 * m, 4 * m + 4):
                ps_v = poolPS.tile([128, H], F32, name="ps_v",
                                   tag="wo" if j % 2 == 0 else "aux", bufs=2)
                for di in range(n_dch):
                    nc.tensor.matmul(ps_v[:], xts[di][:, j * SCH:(j + 1) * SCH],
                                     wvs[di][:], start=(di == 0),
                                     stop=(di == n_dch - 1))
                nc.vector.tensor_copy(v_sb[j][:], ps_v[:])
            for (top_dst, bot_dst, w_tiles, col0) in (
                    (ktop, kbot, wks, 0),
                    (qtop[0], qbot[0], wqs, 0),
                    (qtop[1], qbot[1], wqs, H)):
                ps_top, ps_bot = emit_proj(w_tiles, col0, m)
                if pending is not None:
                    emit_rope(pending)
                pending = (top_dst, bot_dst, m, ps_top, ps_bot)
            if early0 and m == 1:
                emit_rope(pending)
                pending = None
                e0j0 = emit_attention(0, TBLK, 0, pool=poolB0, enc=enc0,
                                      small=True)
                emit_norm(e0j0)
                e0j1 = emit_attention(0, TBLK, 1, pool=poolB0, enc=enc0,
                                      small=True)
                emit_norm(e0j1)
            if early0 and m == 2:
                emit_wo_rs(0, TBLK, pool=poolB0, enc=enc0, small=True)
        if pending is not None:
            emit_rope(pending)

        poolW.release()
        poolB = tc.alloc_tile_pool(name="phaseB", bufs=1)
        if masked:
            poolG = tc.alloc_tile_pool(name="gmask", bufs=4)
        encT = [poolB.tile([128, t], BF, name=f"enc{k}") for k in range(4)]

        if early0:
            regions = regions[1:]
        wo_pending = None

        def norm_pending():
            emit_norm(wo_pending[2][0])
            emit_norm(wo_pending[2][1])

        def flush_pending():
            emit_wo_rs(wo_pending[0], wo_pending[1])

        for (t0, tw) in regions:
            hooks = ({1: norm_pending, 5: flush_pending}
                     if wo_pending is not None else None)
            j0 = emit_attention(t0, tw, 0, mid_hook=hooks)
            j1 = emit_attention(t0, tw, 1)
            wo_pending = (t0, tw, [j0, j1])
        norm_pending()
        flush_pending()
        tc.no_sync_barrier()
        for (t0, tw) in rs_done:
            nc.gpsimd.dma_start(
                out_e.ap()[t0 // 4:(t0 + tw) // 4, :], out_bounces[t0][:])

        if masked:
            poolG.release()
        poolB.release()
        poolB0.release()
        poolD.release()
        poolPS.release()
        poolT.release()
        poolP.release()

    nc.compile()
    return nc


_NC_CACHE = {}


def _get_nc(variant, t=T, d=D):
    key = (variant, t, d)
    if key not in _NC_CACHE:
        if variant == "causal":
            _NC_CACHE[key] = build_fast()
        else:
            _NC_CACHE[key] = build(variant, t, d)
    return _NC_CACHE[key]


def _rope_tables(pos):
    half = H // 2
    freq_exp = (2.0 / H) * np.arange(half, dtype=np.float32)
    timescale = (MAX_WAVELENGTH ** freq_exp).astype(np.float32)
    radians = pos[None, :].astype(np.float32) / timescale[:, None]
    return np.cos(radians).astype(np.float32), np.sin(radians).astype(np.float32)


def _causal_tiles():
    ds = np.arange(SCH)[:, None]
    dt = np.arange(TBLK)[None, :]
    tiles = [(dt >= ds + SCH * i).astype(np.float32) for i in range(4)]
    return np.concatenate(tiles, axis=1).astype(ml_dtypes.bfloat16)


def _prep_in_maps(x, positions, attn_mask, wq, wkv, wo, variant):
    if variant == "causal":
        return _prep_fast_in_maps(x, positions, wq, wkv, wo)
    causal = False
    bf = ml_dtypes.bfloat16
    scale = np.float32(H) ** np.float32(-0.5)
    wq_s = (np.asarray(wq, np.float32) * scale)
    wk = np.asarray(wkv[0, 0], np.float32).astype(bf)
    wv = np.asarray(wkv[1, 0], np.float32).astype(bf)

    in_maps = []
    for c in range(N_CORES):
        b, r = divmod(c, 4)
        h0, h1 = 2 * r, 2 * r + 1
        xT = np.ascontiguousarray(np.asarray(x[b], np.float32).T).astype(bf)
        wq_c = np.ascontiguousarray(
            np.concatenate([wq_s[h0], wq_s[h1]], axis=1)).astype(bf)
        wo_c = np.ascontiguousarray(
            np.concatenate([np.asarray(wo[h0], np.float32),
                            np.asarray(wo[h1], np.float32)], axis=0)).astype(bf)
        cosT, sinT = _rope_tables(np.asarray(positions[b], np.float32))
        m = {"xT": xT, "wq": wq_c, "wk": wk, "wv": wv, "wo": wo_c,
             "cosT": cosT, "sinT": sinT}
        if variant == "masked":
            m["gmask"] = np.ascontiguousarray(
                np.asarray(attn_mask[b, 0], np.float32).T).astype(bf)
        in_maps.append(m)
    return in_maps


def kernel(x, positions, attn_mask, wq, wkv, wo):
    x = np.asarray(x)
    positions = np.asarray(positions)
    attn_mask = np.asarray(attn_mask)
    wq, wkv, wo = np.asarray(wq), np.asarray(wkv), np.asarray(wo)

    tril = np.tril(np.ones((T, T), bool))
    if all(np.array_equal(attn_mask[b, 0], tril) for b in range(B)):
        variant = "causal"
    elif attn_mask.all():
        variant = "dense"
    else:
        variant = "masked"

    nc = _get_nc(variant)
    in_maps = _prep_in_maps(x, positions, attn_mask, wq, wkv, wo, variant)
    res = bass_utils.run_bass_kernel_spmd(nc, in_maps,
                                          core_ids=list(range(N_CORES)))

    if variant == "causal":
        return _unshard_fast(res.results)

    out = np.empty((B, T, D), np.float32)
    for c in range(N_CORES):
        b, r = divmod(c, 4)
        shard = np.asarray(res.results[c]["out"], dtype=np.float32)
        for (t0, tw) in rs_regions(T):
            rows = tw // 4
            out[b, t0 + r * rows:t0 + (r + 1) * rows, :] = \
                shard[t0 // 4:t0 // 4 + rows, :]
    return out
